# revision 1
# baseline (speedup 1.0000x reference)
"""Trainium2 Bass kernel for nn_NisuyNN_90434831384984.

Math: the reference's stack+reshape makes MLP row (s,t,b) depend only on s
(b in {0,1}) or only on t (b in {2,3}), and rows for b=2,3 equal those for
b=0,1 — so the 4096-row x 6-layer MLP collapses to 64 unique rows producing
64 unique 32x32 policy matrices.  The 50-step power iteration (eigengap
~0.012) is replaced by 8 unnormalized steps (converged below fp32 eps; the
final deltas use only intra-vector ratios, so the scale cancels).

Distribution: Megatron-style column-split of every layer across 8 cores,
with per-layer AllGathers of the locally transposed activation slice.
Each layer's output columns are split into G=2 halves so each half's
AllGather overlaps the other half's matmuls (and the next layer starts
on gathered half-0 K-chunks while half-1 is still in flight).  Weights
are sliced on the host and streamed as bf16; PSUM accumulation is fp32;
the eigensolve + deltas tail stays fp32.
"""

import numpy as np

DIM = 128
N = 32
B = 4
H = 4096
NC = 8          # cores
SL = H // NC    # 512 hidden slice per core
OF = N * N      # 1024 output features
OSL = OF // NC  # 128 output slice per core
R = 64          # unique MLP rows
KC = 128        # contraction chunk
G = 1           # column groups per layer (G>1 pipelines AGs but doubles
                # the per-collective CC-core serial floor; G=1 measured best)
PI_ITERS = 7    # extra matvec iterations after the init row-sum step
SLOPE = 0.01

_COMPILED = None
LAST_RESULTS = None


def _build_body(nc, tc, tile, mybir, aps):
    f32 = mybir.dt.float32
    bf16 = mybir.dt.bfloat16
    AF = mybir.ActivationFunctionType
    ALU = mybir.AluOpType
    AX = mybir.AxisListType
    rg = [list(range(NC))]

    from contextlib import ExitStack
    es = ExitStack()
    cpool = es.enter_context(tc.tile_pool(name="consts", bufs=1))
    wpool = es.enter_context(tc.tile_pool(name="w", bufs=20))
    bpool = es.enter_context(tc.tile_pool(name="b", bufs=2))
    apool = es.enter_context(tc.tile_pool(name="act", bufs=2))
    atp = es.enter_context(tc.tile_pool(name="atT", bufs=2))
    lpool = es.enter_context(tc.tile_pool(name="lhs", bufs=3))
    pipool = es.enter_context(tc.tile_pool(name="pi", bufs=2))
    tailp = es.enter_context(tc.tile_pool(name="tail", bufs=1))
    ps = es.enter_context(tc.tile_pool(name="ps", bufs=3, space="PSUM"))
    pst = es.enter_context(tc.tile_pool(name="pst", bufs=2, space="PSUM"))
    dram = es.enter_context(tc.tile_pool(name="dram", bufs=3, space="DRAM"))

    # ---- constants ----
    id64 = cpool.tile([64, 64], bf16)
    nc.gpsimd.dma_start(id64[:], aps["ID64"][:])
    dmask = cpool.tile([R, N], f32)
    nc.gpsimd.dma_start(dmask[:], aps["DMASK"][:])
    t01 = cpool.tile([R, N], f32)
    nc.gpsimd.dma_start(t01[:], aps["T01"][:])
    tt23 = cpool.tile([R, N], f32)
    nc.gpsimd.dma_start(tt23[:], aps["TT23"][:])
    mac = cpool.tile([R, 2], f32)
    nc.gpsimd.dma_start(mac[:], aps["MAC"][:])
    ones = cpool.tile([1, R], bf16)
    nc.vector.memset(ones[:], 1.0)

    # Warm up the collective path: the first collective on silicon pays a
    # ~50us one-time init; absorb it behind the initial weight DMAs with a
    # tiny dummy AllGather whose result feeds an (ignored) external output.
    warm_sb = cpool.tile([KC, 8], bf16)
    nc.vector.memset(warm_sb[:], 0.0)
    warm_in = dram.tile([KC, 8], bf16, tag="warm_in")
    nc.gpsimd.dma_start(warm_in[:], warm_sb[:])
    warm_out = dram.tile([NC * KC, 8], bf16, tag="warm_out", addr_space="Shared")
    nc.gpsimd.collective_compute(
        "AllGather", ALU.bypass, replica_groups=rg,
        ins=[warm_in[:].opt()], outs=[warm_out[:].opt()],
    )
    nc.gpsimd.dma_start(aps["warm"][:], warm_out[0:1, :])

    WCH = 4  # K-chunks per weight tile (0.5 MB pieces keep the DMA path
             # available for the latency-critical gather transfers)

    def load_w(w_ap, nk, width):
        """Stream [nk*128, width] weights as ceil(nk/WCH) chunk tiles."""
        wtiles = []
        for wc in range(0, nk, WCH):
            n = min(WCH, nk - wc)
            wt = wpool.tile([KC, n * width], bf16, tag="w")
            nc.sync.dma_start(
                wt[:].rearrange("p (c n) -> p c n", n=width),
                w_ap[wc * KC:(wc + n) * KC, :].rearrange(
                    "(c p) n -> p c n", p=KC),
            )
            wtiles.append(wt)
        return wtiles

    def half_open(parts, wtiles, btile, width, hw, g, bofs=0):
        """Accumulate output columns [g*hw, (g+1)*hw) over all K-chunk parts.
        parts: list of (lhs_tile, ks) with chunk i of lhs_tile having global
        K-chunk index ks[i] into the chunked weight tiles.  K-chunks
        alternate between the two 64-wide PE column groups (tile_position)
        so two matmuls stream concurrently; psum rows [0:64] and [64:128]
        hold the two partial sums.  Returns the [128, hw] psum tile."""
        pt = ps.tile([2 * R, hw], f32, tag="ps")
        first = [True, True]
        cnt = 0
        n_by_half = [0, 0]
        tot = sum(len(ks) for _, ks in parts)
        for lhs, ks in parts:
            for i, k in enumerate(ks):
                h = cnt % 2
                n_by_half[h] += 1
                is_last_of_h1 = (h == 1) and (n_by_half[1] == tot // 2)
                wt = wtiles[k // WCH]
                kc = k % WCH
                nc.tensor.matmul(
                    pt[h * R:(h + 1) * R, :],
                    lhs[:, i * R:(i + 1) * R],
                    wt[:, kc * width + g * hw: kc * width + (g + 1) * hw],
                    start=first[h],
                    stop=is_last_of_h1,
                    tile_position=(0, h * R),
                    skip_group_check=True,
                )
                first[h] = False
                cnt += 1
        nc.tensor.matmul(
            pt[0:R, :], ones[0:1, :], btile[0:1, bofs:bofs + hw],
            start=first[0], stop=True, tile_position=(0, 0),
            skip_group_check=True,
        )
        return pt

    def sum_halves(pt, hw):
        h0 = apool.tile([R, hw], f32, tag="h0")
        nc.scalar.activation(h0[:], pt[0:R, :], AF.Copy)
        z = apool.tile([R, hw], f32, tag="zsum")
        nc.vector.tensor_tensor(z[:], h0[:], pt[R:2 * R, :], op=ALU.add)
        return z

    def lrelu_half(pt, hw):
        z = sum_halves(pt, hw)
        sc = apool.tile([R, hw], f32, tag="lrelu_sc")
        nc.vector.tensor_scalar_mul(sc[:], z[:], SLOPE)
        act = apool.tile([R, hw], bf16, tag="act")
        nc.vector.tensor_tensor(act[:], z[:], sc[:], op=ALU.max)
        return act

    agin_insts = []

    def gather_half(act, width, hw, g):
        """Transpose the [R, hw] half, AllGather; return (lt, ks) for the
        next layer.  Global next-layer K-chunk for (rank, j): f = rank*width
        + g*hw + j*128."""
        nj = hw // KC
        att = atp.tile([KC, nj * R], bf16, tag="atT")
        for j in range(nj):
            tp = pst.tile([KC, R], bf16, tag="pst")
            nc.tensor.transpose(tp[:], act[:, j * KC:(j + 1) * KC], id64[:])
            nc.vector.tensor_copy(att[:, j * R:(j + 1) * R], tp[:])
        ag_in = dram.tile([KC, nj * R], bf16, tag="agin")
        agin_insts.append(nc.scalar.dma_start(ag_in[:], att[:]))
        ag_out = dram.tile([NC * KC, nj * R], bf16, tag="agout",
                           addr_space="Shared")
        nc.gpsimd.collective_compute(
            "AllGather", ALU.bypass, replica_groups=rg,
            ins=[ag_in[:].opt()], outs=[ag_out[:].opt()],
        )
        lt = lpool.tile([KC, NC * nj * R], bf16, tag="lhs")
        nc.scalar.dma_start(
            lt[:].rearrange("p (r j q) -> p r j q", r=NC, q=R),
            ag_out[:].rearrange("(r p) (j q) -> p r j q", p=KC, q=R),
        )
        ks = [rank * (width // KC) + (g * hw) // KC + j
              for rank in range(NC) for j in range(nj)]
        return [(lt, ks)]

    # ---- layer 1 (K = 256 = 2 chunks, input replicated) ----
    xt = lpool.tile([KC, 2 * R], bf16, tag="lhs")
    nc.sync.dma_start(
        xt[:].rearrange("p (c r) -> p c r", r=R),
        aps["XT"].rearrange("(c p) r -> p c r", p=KC),
    )
    parts = [(xt, [0, 1])]

    for li in range(1, 6):
        width = SL
        nk = (2 * DIM if li == 1 else H) // KC
        hw = width // G
        wtiles = load_w(aps[f"W{li}"], nk, width)
        btile = bpool.tile([1, width], bf16, tag="bias")
        nc.sync.dma_start(btile[0:1, :], aps[f"b{li}"].unsqueeze(0))
        new_parts = []
        for g in range(G):
            pt = half_open(parts, wtiles, btile, width, hw, g)
            act = lrelu_half(pt, hw)
            new_parts.extend(gather_half(act, width, hw, g))
        parts = new_parts

    # ---- layer 6: full W6 on every core (no collective), K=4096, N=1024 ----
    # lrelu -> sigmoid -> mult/add straight into the policy-matrix tile M
    nk6 = H // KC
    b6tile = bpool.tile([1, OF], bf16, tag="bias")
    nc.sync.dma_start(b6tile[0:1, :], aps["b6"].unsqueeze(0))
    M = pipool.tile([R, OF], f32, tag="M")
    for nb in range(2):
        w6tiles = load_w(aps["W6"][:, nb * 512:(nb + 1) * 512], nk6, 512)
        pt = half_open(parts, w6tiles, b6tile, 512, 512, 0, bofs=nb * 512)
        z = sum_halves(pt, 512)
        sc = apool.tile([R, 512], f32, tag="lrelu_sc")
        nc.vector.tensor_scalar_mul(sc[:], z[:], SLOPE)
        lr = apool.tile([R, 512], f32, tag="lrelu_out")
        nc.vector.tensor_tensor(lr[:], z[:], sc[:], op=ALU.max)
        sg = apool.tile([R, 512], f32, tag="sig")
        nc.scalar.activation(sg[:], lr[:], AF.Sigmoid)
        nc.vector.tensor_scalar(
            M[:, nb * 512:(nb + 1) * 512], sg[:], mac[:, 0:1], mac[:, 1:2],
            op0=ALU.mult, op1=ALU.add,
        )

    # ---- power iteration: b <- M b, unnormalized ----
    # early iterations multiply in bf16 (self-correcting; only the final
    # step's precision survives), last iteration in fp32
    M3 = M[:].rearrange("p (r q) -> p r q", q=N)
    Mb = pipool.tile([R, OF], bf16, tag="Mb")
    nc.scalar.activation(Mb[:], M[:], AF.Copy)
    Mb3 = Mb[:].rearrange("p (r q) -> p r q", q=N)
    bv = pipool.tile([R, N], f32, tag="bv")
    nc.vector.reduce_sum(bv[:], M3, axis=AX.X)  # first step from b0 = ones
    for it in range(PI_ITERS):
        last = it == PI_ITERS - 1
        bb = bv[:].unsqueeze(1).broadcast_to((R, N, N))
        if last:
            tmp = pipool.tile([R, OF], f32, tag="pit")
            t3 = tmp[:].rearrange("p (r q) -> p r q", q=N)
            nc.vector.tensor_tensor(t3, M3, bb, op=ALU.mult)
        else:
            tmp = pipool.tile([R, OF], bf16, tag="pitb")
            t3 = tmp[:].rearrange("p (r q) -> p r q", q=N)
            nc.vector.tensor_tensor(t3, Mb3, bb, op=ALU.mult)
        bv = pipool.tile([R, N], f32, tag="bv")
        nc.vector.reduce_sum(bv[:], t3, axis=AX.X)

    # ---- deltas tail ----
    scr = tailp.tile([R, N], f32, tag="scr")
    d = tailp.tile([R, 1], f32, tag="d")
    nc.vector.tensor_tensor(scr[:], bv[:], dmask[:], op=ALU.mult)
    nc.vector.reduce_sum(d[:], scr[:], axis=AX.X)
    recipd = tailp.tile([R, 1], f32, tag="rd")
    nc.vector.reciprocal(recipd[:], d[:])
    recipE = tailp.tile([R, N], f32, tag="rE")
    nc.vector.reciprocal(recipE[:], bv[:])
    w01 = tailp.tile([R, 1], f32, tag="w01")
    nc.vector.reduce_sum(w01[:], t01[:], axis=AX.X)
    coef_s = tailp.tile([R, 1], f32, tag="cs")
    nc.vector.tensor_tensor(coef_s[:], w01[:], recipd[:], op=ALU.mult)
    scr2 = tailp.tile([R, N], f32, tag="scr2")
    c23 = tailp.tile([R, 1], f32, tag="c23")
    nc.vector.tensor_tensor(scr2[:], tt23[:], recipE[:], op=ALU.mult)
    nc.vector.reduce_sum(c23[:], scr2[:], axis=AX.X)
    coef = tailp.tile([R, B], f32, tag="coef")
    nc.vector.memset(coef[:], 0.0)
    nc.vector.tensor_copy(coef[0:32, 0:1], coef_s[0:32, :])
    nc.vector.tensor_copy(coef[32:64, 1:2], coef_s[32:64, :])
    nc.vector.tensor_copy(coef[0:32, 2:3], c23[0:32, :])
    nc.vector.tensor_copy(coef[32:64, 3:4], c23[32:64, :])
    pd = pst.tile([B, N], f32, tag="pd")
    nc.tensor.matmul(pd[:], coef[:], bv[:], start=True, stop=True)
    osb = tailp.tile([B, N], f32, tag="osb")
    nc.vector.tensor_copy(osb[:], pd[:])
    nc.sync.dma_start(aps["out"][:], osb[:])
    es.close()


def build():
    import concourse.bacc as bacc
    import concourse.mybir as mybir
    import concourse.tile as tile

    f32 = mybir.dt.float32
    bf16 = mybir.dt.bfloat16
    nc = bacc.Bacc("TRN2", target_bir_lowering=False, debug=False, num_devices=NC)
    shapes = {
        "XT": ([2 * DIM, R], bf16),
        "W1": ([2 * DIM, SL], bf16), "b1": ([SL], bf16),
        "W2": ([H, SL], bf16), "b2": ([SL], bf16),
        "W3": ([H, SL], bf16), "b3": ([SL], bf16),
        "W4": ([H, SL], bf16), "b4": ([SL], bf16),
        "W5": ([H, SL], bf16), "b5": ([SL], bf16),
        "W6": ([H, OF], bf16), "b6": ([OF], bf16),
        "T01": ([R, N], f32), "TT23": ([R, N], f32),
        "DMASK": ([R, N], f32), "MAC": ([R, 2], f32), "ID64": ([64, 64], bf16),
    }
    aps = {
        k: nc.dram_tensor(k, v[0], v[1], kind="ExternalInput").ap()
        for k, v in shapes.items()
    }
    aps["out"] = nc.dram_tensor("out", [B, N], f32, kind="ExternalOutput").ap()
    aps["warm"] = nc.dram_tensor("warm", [1, 8], bf16, kind="ExternalOutput").ap()
    with tile.TileContext(nc) as tc:
        _build_body(nc, tc, tile, mybir, aps)
    nc.compile()
    return nc


def prep_in_maps(inputs):
    import ml_dtypes
    f = np.float32
    bf = ml_dtypes.bfloat16
    E = np.asarray(inputs["batch_node_embeddings"], f)   # (B,N,D)
    T = np.asarray(inputs["batch_Ts"], f)                # (B,N,N)
    mult = np.asarray(inputs["mult_const_batch"], f).reshape(-1)[0]
    add = np.asarray(inputs["add_const_batch"], f).reshape(-1)[0]
    S = np.transpose(E, (1, 0, 2))                       # (N,B,D)
    G0 = np.concatenate([S[:, 0], S[:, 1]], axis=-1)     # (N, 2D)
    G1 = np.concatenate([S[:, 2], S[:, 3]], axis=-1)
    rows = np.concatenate([G0, G1], axis=0)              # (64, 256)
    common = {
        "XT": np.ascontiguousarray(rows.T).astype(bf),
        "T01": np.ascontiguousarray(np.concatenate([T[0], T[1]], axis=0)),
        "TT23": np.ascontiguousarray(np.concatenate([T[2].T, T[3].T], axis=0)),
        "DMASK": np.ascontiguousarray(np.tile(np.eye(N, dtype=f), (2, 1))),
        "MAC": np.ascontiguousarray(
            np.stack([np.full(R, mult, f), np.full(R, add, f)], axis=1)
        ),
        "ID64": np.eye(64, dtype=bf),
    }
    in_maps = []
    for c in range(NC):
        m = dict(common)
        for li in range(1, 6):
            W = np.asarray(inputs[f"W{li}"], f)
            b = np.asarray(inputs[f"b{li}"], f)
            m[f"W{li}"] = np.ascontiguousarray(W[:, c * SL:(c + 1) * SL]).astype(bf)
            m[f"b{li}"] = np.ascontiguousarray(b[c * SL:(c + 1) * SL]).astype(bf)
        m["W6"] = np.asarray(inputs["W6"], f).astype(bf)
        m["b6"] = np.asarray(inputs["b6"], f).astype(bf)
        in_maps.append(m)
    return in_maps


def kernel(**inputs):
    global _COMPILED, LAST_RESULTS
    from concourse import bass_utils

    if _COMPILED is None:
        _COMPILED = build()
    in_maps = prep_in_maps(inputs)
    res = bass_utils.run_bass_kernel_spmd(
        _COMPILED, in_maps, core_ids=list(range(NC))
    )
    LAST_RESULTS = res
    return np.asarray(res.results[0]["out"], np.float32)



# revision 12
# speedup vs baseline: 1.2997x; 1.2997x over previous
"""Trainium2 Bass kernel for nn_NisuyNN_90434831384984.

Math: the reference's stack+reshape makes MLP row (s,t,b) depend only on s
(b in {0,1}) or only on t (b in {2,3}), and rows for b=2,3 equal those for
b=0,1 — so the 4096-row x 6-layer MLP collapses to 64 unique rows producing
64 unique 32x32 policy matrices.  The 50-step power iteration has a large
eigengap; 3 unnormalized extra steps after the row-sum init reach the bf16
noise floor (the final deltas use only intra-vector ratios, so scale
cancels).

Distribution: layer 1 is fully replicated on all 8 cores (W1 is only 2MB;
this removes the first AllGather so the one-time ~60us collectives-init
barrier hides behind layers 1-2 compute).  Layers 2-5 are Megatron
column-split with a per-layer AllGather of the locally transposed
activation slice.  Layer 6 keeps the full W6 on every core (policy matrix
fully local; no collective), then the power-iteration + deltas tail runs
replicated and core 0's output is returned.

Weights are host-prelayouted chunk-major ([128, nk*width], chunk = 128
contiguous K rows) so every weight DMA moves 8KB-contiguous lines per
partition instead of 1KB gather segments.
"""

import numpy as np

DIM = 128
N = 32
B = 4
H = 4096
NC = 8          # cores
SL = H // NC    # 512 hidden slice per core
OF = N * N      # 1024 output features
R = 64          # unique MLP rows
KC = 128        # contraction chunk
TK = 8          # K-chunks per weight tile
PI_ITERS = 3    # extra matvec iterations after the init row-sum step
SLOPE = 0.01

_COMPILED = None
LAST_RESULTS = None


def _build_body(nc, tc, tile, mybir, aps):
    f32 = mybir.dt.float32
    bf16 = mybir.dt.bfloat16
    AF = mybir.ActivationFunctionType
    ALU = mybir.AluOpType
    AX = mybir.AxisListType
    rg = [list(range(NC))]

    from contextlib import ExitStack
    es = ExitStack()
    cpool = es.enter_context(tc.tile_pool(name="consts", bufs=1))
    wpool = es.enter_context(tc.tile_pool(name="w", bufs=14))
    bpool = es.enter_context(tc.tile_pool(name="b", bufs=2))
    apool = es.enter_context(tc.tile_pool(name="act", bufs=2))
    atp = es.enter_context(tc.tile_pool(name="atT", bufs=2))
    lpool = es.enter_context(tc.tile_pool(name="lhs", bufs=3))
    pipool = es.enter_context(tc.tile_pool(name="pi", bufs=2))
    tailp = es.enter_context(tc.tile_pool(name="tail", bufs=1))
    ps = es.enter_context(tc.tile_pool(name="ps", bufs=3, space="PSUM"))
    pst = es.enter_context(tc.tile_pool(name="pst", bufs=4, space="PSUM"))
    dram = es.enter_context(tc.tile_pool(name="dram", bufs=3, space="DRAM"))

    # Warm up the collective path first: the first collective pays a ~50us
    # one-time init barrier; trigger it immediately (input is host-provided
    # so the doorbell fires as soon as the gpsimd preamble ends) and absorb
    # it behind layers 1-2, which need no collective.
    warm_in = dram.tile([KC, 8], bf16, tag="warm_in")
    nc.gpsimd.dma_start(warm_in[:], aps["WARMIN"][:])
    warm_out = dram.tile([NC * KC, 8], bf16, tag="warm_out", addr_space="Shared")
    nc.gpsimd.collective_compute(
        "AllGather", ALU.bypass, replica_groups=rg,
        ins=[warm_in[:].opt()], outs=[warm_out[:].opt()],
    )

    # ---- constants ----
    id64 = cpool.tile([64, 64], bf16)
    nc.gpsimd.dma_start(id64[:], aps["ID64"][:])
    dmask = cpool.tile([R, N], f32)
    nc.gpsimd.dma_start(dmask[:], aps["DMASK"][:])
    t01 = cpool.tile([R, N], f32)
    nc.gpsimd.dma_start(t01[:], aps["T01"][:])
    tt23 = cpool.tile([R, N], f32)
    nc.gpsimd.dma_start(tt23[:], aps["TT23"][:])
    mac = cpool.tile([R, 2], f32)
    nc.gpsimd.dma_start(mac[:], aps["MAC"][:])
    ones = cpool.tile([1, R], bf16)
    nc.vector.memset(ones[:], 1.0)
    # keep the warm AG live via a tiny output DMA (gpsimd, like baseline)
    nc.gpsimd.dma_start(aps["warm"][:], warm_out[0:1, :])

    def load_w(w_ap, nk, ncols):
        """Stream [128, nk*ncols] chunk-major weights as contiguous tiles
        holding TK chunks each (ncols=512 -> 8KB per partition line)."""
        wtiles = []
        for t in range(0, nk, TK):
            n = min(TK, nk - t)
            wt = wpool.tile([KC, n * ncols], bf16, tag="w")
            nc.sync.dma_start(wt[:], w_ap[:, t * ncols:(t + n) * ncols])
            wtiles.append(wt)
        return wtiles

    def mm_layer(pt, lhs_of, wt_of, nk, btile, bofs, bw):
        """Accumulate psum [128, bw]: rows 0:64 = even K-chunks (+bias),
        rows 64:128 = odd K-chunks; the two PE column groups pipeline."""
        first = [True, True]
        n_by_half = [0, 0]
        for k in range(nk):
            h = k % 2
            n_by_half[h] += 1
            is_last_h1 = (h == 1) and (n_by_half[1] == nk // 2)
            nc.tensor.matmul(
                pt[h * R:(h + 1) * R, :],
                lhs_of(k),
                wt_of(k),
                start=first[h],
                stop=is_last_h1,
                tile_position=(0, h * R),
                skip_group_check=True,
            )
            first[h] = False
        nc.tensor.matmul(
            pt[0:R, :], ones[0:1, :], btile[0:1, bofs:bofs + bw],
            start=first[0], stop=True, tile_position=(0, 0),
            skip_group_check=True,
        )

    def sum_halves(pt, hw):
        """z = psum_rows0 + psum_rows64 (DVE can read only one PSUM input)."""
        h0 = apool.tile([R, hw], f32, tag="h0")
        nc.scalar.activation(h0[:], pt[0:R, :], AF.Copy)
        z = apool.tile([R, hw], f32, tag="zsum")
        nc.vector.tensor_tensor(z[:], h0[:], pt[R:2 * R, :], op=ALU.add)
        return z

    def lrelu_act(pt, hw):
        """act_bf16 = leaky_relu(psum_rows0 + psum_rows64)."""
        z = sum_halves(pt, hw)
        act = apool.tile([R, hw], bf16, tag="act")
        nc.scalar.activation(act[:], z[:], AF.Lrelu, alpha=SLOPE)
        return act

    # ---- layer 1, fully replicated: [64, 256] @ [256, 4096] ----
    xt = lpool.tile([KC, 2 * R], bf16, tag="xt", bufs=1)
    nc.sync.dma_start(xt[:], aps["XT"][:])
    w1t = wpool.tile([KC, 2 * H], bf16, tag="w1", bufs=1)  # both K-chunks
    nc.sync.dma_start(w1t[:], aps["W1"][:])
    b1t = bpool.tile([1, H], bf16, tag="b1", bufs=1)
    nc.sync.dma_start(b1t[0:1, :], aps["b1"].unsqueeze(0))

    lt = lpool.tile([KC, H // KC * R], bf16, tag="lhs")  # [128, 2048]
    for blk in range(8):
        pt = ps.tile([2 * R, SL], f32, tag="ps")
        mm_layer(
            pt,
            lambda k: xt[:, k * R:(k + 1) * R],
            lambda k: w1t[:, k * H + blk * SL:k * H + (blk + 1) * SL],
            2, b1t, blk * SL, SL,
        )
        act = lrelu_act(pt, SL)
        for j in range(4):
            tp = pst.tile([KC, R], bf16, tag="pst")
            nc.tensor.transpose(tp[:], act[:, j * KC:(j + 1) * KC], id64[:])
            dst = lt[:, (blk * 4 + j) * R:(blk * 4 + j + 1) * R]
            if j % 2 == 0:
                nc.vector.tensor_copy(dst, tp[:])
            else:
                nc.scalar.activation(dst, tp[:], AF.Copy)

    # ---- layers 2-5: Megatron column split, AllGather per layer ----
    for li in range(2, 6):
        nk = H // KC
        wtiles = load_w(aps[f"W{li}"], nk, SL)
        btile = bpool.tile([1, SL], bf16, tag="bias")
        nc.sync.dma_start(btile[0:1, :], aps[f"b{li}"].unsqueeze(0))
        pt = ps.tile([2 * R, SL], f32, tag="ps")
        mm_layer(
            pt,
            lambda k: lt[:, k * R:(k + 1) * R],
            lambda k: wtiles[k // TK][:, (k % TK) * SL:(k % TK + 1) * SL],
            nk, btile, 0, SL,
        )
        act = lrelu_act(pt, SL)
        # transpose the [64, 512] slice to [128, 4*64] and AllGather it
        att = atp.tile([KC, 4 * R], bf16, tag="atT")
        for j in range(4):
            tp = pst.tile([KC, R], bf16, tag="pst")
            nc.tensor.transpose(tp[:], act[:, j * KC:(j + 1) * KC], id64[:])
            dst = att[:, j * R:(j + 1) * R]
            if j % 2 == 0:
                nc.vector.tensor_copy(dst, tp[:])
            else:
                nc.scalar.activation(dst, tp[:], AF.Copy)
        ag_in = dram.tile([KC, 4 * R], bf16, tag="agin")
        nc.scalar.dma_start(ag_in[:], att[:])
        ag_out = dram.tile([NC * KC, 4 * R], bf16, tag="agout",
                           addr_space="Shared")
        nc.gpsimd.collective_compute(
            "AllGather", ALU.bypass, replica_groups=rg,
            ins=[ag_in[:].opt()], outs=[ag_out[:].opt()],
        )
        # per-rank contiguous loads back to SBUF, spread across queues
        lt = lpool.tile([KC, NC * 4 * R], bf16, tag="lhs")
        for r in range(NC):
            src = ag_out[r * KC:(r + 1) * KC, :]
            dst = lt[:, r * 4 * R:(r + 1) * 4 * R]
            eng = (nc.scalar, nc.sync, nc.gpsimd)[r % 3]
            eng.dma_start(dst, src)

    # ---- layer 6: full W6 on every core (no collective), K=4096, N=1024 ----
    nk6 = H // KC
    b6t = bpool.tile([1, OF], bf16, tag="b6")
    nc.sync.dma_start(b6t[0:1, :], aps["b6"].unsqueeze(0))
    M = pipool.tile([R, OF], f32, tag="M")
    for nb in range(2):
        w6tiles = load_w(aps[f"W6{'ab'[nb]}"], nk6, SL)
        pt = ps.tile([2 * R, SL], f32, tag="ps")
        mm_layer(
            pt,
            lambda k: lt[:, k * R:(k + 1) * R],
            lambda k: w6tiles[k // TK][:, (k % TK) * SL:(k % TK + 1) * SL],
            nk6, b6t, nb * SL, SL,
        )
        z = sum_halves(pt, SL)
        lr = apool.tile([R, SL], f32, tag="lrelu_out")
        nc.scalar.activation(lr[:], z[:], AF.Lrelu, alpha=SLOPE)
        sg = apool.tile([R, SL], f32, tag="sig")
        nc.scalar.activation(sg[:], lr[:], AF.Sigmoid)
        nc.vector.tensor_scalar(
            M[:, nb * SL:(nb + 1) * SL], sg[:], mac[:, 0:1], mac[:, 1:2],
            op0=ALU.mult, op1=ALU.add,
        )

    # ---- power iteration: b <- M b, unnormalized ----
    M3 = M[:].rearrange("p (r q) -> p r q", q=N)
    Mb = pipool.tile([R, OF], bf16, tag="Mb")
    nc.scalar.activation(Mb[:], M[:], AF.Copy)
    Mb3 = Mb[:].rearrange("p (r q) -> p r q", q=N)
    bv = pipool.tile([R, N], f32, tag="bv")
    nc.vector.reduce_sum(bv[:], M3, axis=AX.X)  # first step from b0 = ones
    for it in range(PI_ITERS):
        last = it == PI_ITERS - 1
        bb = bv[:].unsqueeze(1).broadcast_to((R, N, N))
        if last:
            tmp = pipool.tile([R, OF], f32, tag="pit")
            t3 = tmp[:].rearrange("p (r q) -> p r q", q=N)
            nc.vector.tensor_tensor(t3, M3, bb, op=ALU.mult)
        else:
            tmp = pipool.tile([R, OF], bf16, tag="pitb")
            t3 = tmp[:].rearrange("p (r q) -> p r q", q=N)
            nc.vector.tensor_tensor(t3, Mb3, bb, op=ALU.mult)
        bv = pipool.tile([R, N], f32, tag="bv")
        nc.vector.reduce_sum(bv[:], t3, axis=AX.X)

    # ---- deltas tail ----
    scr = tailp.tile([R, N], f32, tag="scr")
    d = tailp.tile([R, 1], f32, tag="d")
    nc.vector.tensor_tensor(scr[:], bv[:], dmask[:], op=ALU.mult)
    nc.vector.reduce_sum(d[:], scr[:], axis=AX.X)
    recipd = tailp.tile([R, 1], f32, tag="rd")
    nc.vector.reciprocal(recipd[:], d[:])
    recipE = tailp.tile([R, N], f32, tag="rE")
    nc.vector.reciprocal(recipE[:], bv[:])
    w01 = tailp.tile([R, 1], f32, tag="w01")
    nc.vector.reduce_sum(w01[:], t01[:], axis=AX.X)
    coef_s = tailp.tile([R, 1], f32, tag="cs")
    nc.vector.tensor_tensor(coef_s[:], w01[:], recipd[:], op=ALU.mult)
    scr2 = tailp.tile([R, N], f32, tag="scr2")
    c23 = tailp.tile([R, 1], f32, tag="c23")
    nc.vector.tensor_tensor(scr2[:], tt23[:], recipE[:], op=ALU.mult)
    nc.vector.reduce_sum(c23[:], scr2[:], axis=AX.X)
    coef = tailp.tile([R, B], f32, tag="coef")
    nc.vector.memset(coef[:], 0.0)
    nc.vector.tensor_copy(coef[0:32, 0:1], coef_s[0:32, :])
    nc.vector.tensor_copy(coef[32:64, 1:2], coef_s[32:64, :])
    nc.vector.tensor_copy(coef[0:32, 2:3], c23[0:32, :])
    nc.vector.tensor_copy(coef[32:64, 3:4], c23[32:64, :])
    pd = pst.tile([B, N], f32, tag="pd", bufs=1)
    nc.tensor.matmul(pd[:], coef[:], bv[:], start=True, stop=True)
    osb = tailp.tile([B, N], f32, tag="osb")
    nc.vector.tensor_copy(osb[:], pd[:])
    nc.sync.dma_start(aps["out"][:], osb[:])
    es.close()


def build():
    import concourse.bacc as bacc
    import concourse.mybir as mybir
    import concourse.tile as tile

    f32 = mybir.dt.float32
    bf16 = mybir.dt.bfloat16
    nc = bacc.Bacc("TRN2", target_bir_lowering=False, debug=False, num_devices=NC)
    shapes = {
        "XT": ([KC, 2 * R], bf16),
        "W1": ([KC, 2 * H], bf16), "b1": ([H], bf16),
        "W2": ([KC, H // KC * SL], bf16), "b2": ([SL], bf16),
        "W3": ([KC, H // KC * SL], bf16), "b3": ([SL], bf16),
        "W4": ([KC, H // KC * SL], bf16), "b4": ([SL], bf16),
        "W5": ([KC, H // KC * SL], bf16), "b5": ([SL], bf16),
        "W6a": ([KC, H // KC * SL], bf16), "W6b": ([KC, H // KC * SL], bf16),
        "b6": ([OF], bf16),
        "T01": ([R, N], f32), "TT23": ([R, N], f32),
        "DMASK": ([R, N], f32), "MAC": ([R, 2], f32), "ID64": ([64, 64], bf16),
        "WARMIN": ([KC, 8], bf16),
    }
    aps = {
        k: nc.dram_tensor(k, v[0], v[1], kind="ExternalInput").ap()
        for k, v in shapes.items()
    }
    aps["out"] = nc.dram_tensor("out", [B, N], f32, kind="ExternalOutput").ap()
    aps["warm"] = nc.dram_tensor("warm", [1, 8], bf16, kind="ExternalOutput").ap()
    with tile.TileContext(nc) as tc:
        _build_body(nc, tc, tile, mybir, aps)
    nc.compile()
    return nc


def _chunk_major(W):
    """[K, width] -> [128, (K//128)*width]; chunk k's rows land contiguous."""
    K, width = W.shape
    nk = K // KC
    return np.ascontiguousarray(
        W.reshape(nk, KC, width).transpose(1, 0, 2).reshape(KC, nk * width)
    )


def prep_in_maps(inputs):
    import ml_dtypes
    f = np.float32
    bf = ml_dtypes.bfloat16
    E = np.asarray(inputs["batch_node_embeddings"], f)   # (B,N,D)
    T = np.asarray(inputs["batch_Ts"], f)                # (B,N,N)
    mult = np.asarray(inputs["mult_const_batch"], f).reshape(-1)[0]
    add = np.asarray(inputs["add_const_batch"], f).reshape(-1)[0]
    S = np.transpose(E, (1, 0, 2))                       # (N,B,D)
    G0 = np.concatenate([S[:, 0], S[:, 1]], axis=-1)     # (N, 2D)
    G1 = np.concatenate([S[:, 2], S[:, 3]], axis=-1)
    rows = np.concatenate([G0, G1], axis=0)              # (64, 256)
    XT = np.ascontiguousarray(rows.T)                    # (256, 64)
    W6 = np.asarray(inputs["W6"], f)
    common = {
        "XT": _chunk_major(XT).astype(bf),
        "W1": _chunk_major(np.asarray(inputs["W1"], f)).astype(bf),
        "b1": np.asarray(inputs["b1"], f).astype(bf),
        "W6a": _chunk_major(W6[:, 0:512]).astype(bf),
        "W6b": _chunk_major(W6[:, 512:1024]).astype(bf),
        "b6": np.asarray(inputs["b6"], f).astype(bf),
        "T01": np.ascontiguousarray(np.concatenate([T[0], T[1]], axis=0)),
        "TT23": np.ascontiguousarray(np.concatenate([T[2].T, T[3].T], axis=0)),
        "DMASK": np.ascontiguousarray(np.tile(np.eye(N, dtype=f), (2, 1))),
        "MAC": np.ascontiguousarray(
            np.stack([np.full(R, mult, f), np.full(R, add, f)], axis=1)
        ),
        "ID64": np.eye(64, dtype=bf),
        "WARMIN": np.zeros((KC, 8), bf),
    }
    in_maps = []
    for c in range(NC):
        m = dict(common)
        for li in range(2, 6):
            W = np.asarray(inputs[f"W{li}"], f)
            b = np.asarray(inputs[f"b{li}"], f)
            m[f"W{li}"] = _chunk_major(W[:, c * SL:(c + 1) * SL]).astype(bf)
            m[f"b{li}"] = np.ascontiguousarray(b[c * SL:(c + 1) * SL]).astype(bf)
        in_maps.append(m)
    return in_maps


def kernel(**inputs):
    global _COMPILED, LAST_RESULTS
    from concourse import bass_utils

    if _COMPILED is None:
        _COMPILED = build()
    in_maps = prep_in_maps(inputs)
    res = bass_utils.run_bass_kernel_spmd(
        _COMPILED, in_maps, core_ids=list(range(NC))
    )
    LAST_RESULTS = res
    return np.asarray(res.results[0]["out"], np.float32)


# revision 22
# speedup vs baseline: 1.4385x; 1.1068x over previous
"""Trainium2 Bass kernel for nn_NisuyNN_90434831384984.

Math: the reference's stack+reshape makes MLP row (s,t,b) depend only on s
(b in {0,1}) or only on t (b in {2,3}), and rows for b=2,3 equal those for
b=0,1 — so the 4096-row x 6-layer MLP collapses to 64 unique rows producing
64 unique 32x32 policy matrices.  The 50-step power iteration has a large
eigengap; 3 unnormalized extra steps after the row-sum init reach the bf16
noise floor (the final deltas use only intra-vector ratios, so scale
cancels).

Distribution: layer 1 is fully replicated on all 8 cores (W1 is only 2MB;
this removes the first AllGather so the one-time ~60us collectives-init
barrier hides behind layers 1-2 compute).  Layers 2-5 are Megatron
column-split with a per-layer AllGather of the locally transposed
activation slice.  Layer 6 keeps the full W6 on every core (policy matrix
fully local; no collective), then the power-iteration + deltas tail runs
replicated and core 0's output is returned.

Weights are host-prelayouted chunk-major ([128, nk*width], chunk = 128
contiguous K rows) so every weight DMA moves 8KB-contiguous lines per
partition instead of 1KB gather segments.
"""

import numpy as np

DIM = 128
N = 32
B = 4
H = 4096
NC = 8          # cores
SL = H // NC    # 512 hidden slice per core
OF = N * N      # 1024 output features
R = 64          # unique MLP rows
KC = 128        # contraction chunk
TK = 8          # K-chunks per weight tile
PI_ITERS = 2    # extra matvec iterations after the init row-sum step
SLOPE = 0.01

_COMPILED = None
LAST_RESULTS = None


def _build_body(nc, tc, tile, mybir, aps):
    f32 = mybir.dt.float32
    bf16 = mybir.dt.bfloat16
    AF = mybir.ActivationFunctionType
    ALU = mybir.AluOpType
    AX = mybir.AxisListType
    rg = [list(range(NC))]

    from contextlib import ExitStack
    es = ExitStack()
    cpool = es.enter_context(tc.tile_pool(name="consts", bufs=1))
    wpool = es.enter_context(tc.tile_pool(name="w", bufs=14))
    bpool = es.enter_context(tc.tile_pool(name="b", bufs=2))
    apool = es.enter_context(tc.tile_pool(name="act", bufs=2))
    atp = es.enter_context(tc.tile_pool(name="atT", bufs=2))
    lpool = es.enter_context(tc.tile_pool(name="lhs", bufs=3))
    pipool = es.enter_context(tc.tile_pool(name="pi", bufs=2))
    tailp = es.enter_context(tc.tile_pool(name="tail", bufs=1))
    ps = es.enter_context(tc.tile_pool(name="ps", bufs=3, space="PSUM"))
    pst = es.enter_context(tc.tile_pool(name="pst", bufs=4, space="PSUM"))
    dram = es.enter_context(tc.tile_pool(name="dram", bufs=3, space="DRAM"))

    # Warm up the collective path first: the first collective pays a ~50us
    # one-time init barrier; trigger it immediately (input is host-provided
    # so the doorbell fires as soon as the gpsimd preamble ends) and absorb
    # it behind layers 1-2, which need no collective.
    warm_in = dram.tile([KC, 8], bf16, tag="warm_in")
    warm_out = dram.tile([NC * KC, 8], bf16, tag="warm_out", addr_space="Shared")
    # warm_in is deliberately never written: the gathered bytes are ignored,
    # and skipping the staging DMA lets the doorbell (and with it the global
    # CC-init barrier that all cores must join) fire as early as possible.
    nc.gpsimd.collective_compute(
        "AllGather", ALU.bypass, replica_groups=rg,
        ins=[warm_in[:].opt()], outs=[warm_out[:].opt()],
    )

    # ---- constants ----
    id64 = cpool.tile([64, 64], bf16)
    nc.gpsimd.dma_start(id64[:], aps["ID64"][:])
    dmask = cpool.tile([R, N], f32)
    nc.gpsimd.dma_start(dmask[:], aps["DMASK"][:])
    t01 = cpool.tile([R, 1], f32)  # host-precomputed row sums of T[0],T[1]
    nc.gpsimd.dma_start(t01[:], aps["T01"][:])
    tt23 = cpool.tile([R, N], f32)
    nc.gpsimd.dma_start(tt23[:], aps["TT23"][:])
    mac = cpool.tile([R, 2], f32)
    nc.gpsimd.dma_start(mac[:], aps["MAC"][:])
    ones = cpool.tile([1, R], bf16)
    nc.vector.memset(ones[:], 1.0)
    # keep the warm AG live via a tiny output DMA (gpsimd, like baseline)
    nc.gpsimd.dma_start(aps["warm"][:], warm_out[0:1, :])

    def load_w(w_ap, nk, ncols):
        """Stream [128, nk*ncols] chunk-major weights as contiguous tiles
        holding TK chunks each (ncols=512 -> 8KB per partition line)."""
        wtiles = []
        for t in range(0, nk, TK):
            n = min(TK, nk - t)
            wt = wpool.tile([KC, n * ncols], bf16, tag="w")
            nc.sync.dma_start(wt[:], w_ap[:, t * ncols:(t + n) * ncols])
            wtiles.append(wt)
        return wtiles

    def mm_layer(pt, lhs_of, wt_of, nk, btile, bofs, bw):
        """Accumulate psum [128, bw]: rows 0:64 = bias + even K-chunks,
        rows 64:128 = odd K-chunks; the two PE column groups pipeline.
        Bias goes first so the even half finishes (and the h0 copy can
        start) before the final odd-half matmul completes."""
        nc.tensor.matmul(
            pt[0:R, :], ones[0:1, :], btile[0:1, bofs:bofs + bw],
            start=True, stop=False, tile_position=(0, 0),
            skip_group_check=True,
        )
        first_h1 = True
        n_by_half = [0, 0]
        for k in range(nk):
            h = k % 2
            n_by_half[h] += 1
            is_last = n_by_half[h] == (nk + 1 - h) // 2
            nc.tensor.matmul(
                pt[h * R:(h + 1) * R, :],
                lhs_of(k),
                wt_of(k),
                start=(h == 1 and first_h1),
                stop=is_last,
                tile_position=(0, h * R),
                skip_group_check=True,
            )
            if h == 1:
                first_h1 = False

    def sum_halves(pt, hw):
        """z = psum_rows0 + psum_rows64 (DVE can read only one PSUM input)."""
        h0 = apool.tile([R, hw], f32, tag="h0")
        nc.scalar.activation(h0[:], pt[0:R, :], AF.Copy)
        z = apool.tile([R, hw], f32, tag="zsum")
        nc.vector.tensor_tensor(z[:], h0[:], pt[R:2 * R, :], op=ALU.add)
        return z

    def lrelu_act(pt, hw):
        """act_bf16 = leaky_relu(psum_rows0 + psum_rows64)."""
        z = sum_halves(pt, hw)
        act = apool.tile([R, hw], bf16, tag="act")
        nc.scalar.activation(act[:], z[:], AF.Lrelu, alpha=SLOPE)
        return act

    # ---- layer 1, fully replicated: [64, 256] @ [256, 4096] ----
    xt = lpool.tile([KC, 2 * R], bf16, tag="xt", bufs=1)
    nc.sync.dma_start(xt[:], aps["XT"][:])
    w1t = wpool.tile([KC, 2 * H], bf16, tag="w1", bufs=1)  # both K-chunks
    nc.sync.dma_start(w1t[:], aps["W1"][:])
    b1t = bpool.tile([1, H], bf16, tag="b1", bufs=1)
    nc.sync.dma_start(b1t[0:1, :], aps["b1"].unsqueeze(0))

    lt = lpool.tile([KC, H // KC * R], bf16, tag="lhs")  # [128, 2048]
    for blk in range(8):
        pt = ps.tile([2 * R, SL], f32, tag="ps")
        mm_layer(
            pt,
            lambda k: xt[:, k * R:(k + 1) * R],
            lambda k: w1t[:, k * H + blk * SL:k * H + (blk + 1) * SL],
            2, b1t, blk * SL, SL,
        )
        act = lrelu_act(pt, SL)
        for j in range(4):
            tp = pst.tile([KC, R], bf16, tag="pst")
            nc.tensor.transpose(tp[:], act[:, j * KC:(j + 1) * KC], id64[:])
            dst = lt[:, (blk * 4 + j) * R:(blk * 4 + j + 1) * R]
            if j % 2 == 0:
                nc.vector.tensor_copy(dst, tp[:])
            else:
                nc.scalar.activation(dst, tp[:], AF.Copy)

    # ---- layers 2-5: Megatron column split, AllGather per layer ----
    for li in range(2, 6):
        nk = H // KC
        wtiles = load_w(aps[f"W{li}"], nk, SL)
        btile = bpool.tile([1, SL], bf16, tag="bias")
        nc.sync.dma_start(btile[0:1, :], aps[f"b{li}"].unsqueeze(0))
        pt = ps.tile([2 * R, SL], f32, tag="ps")
        mm_layer(
            pt,
            lambda k: lt[:, k * R:(k + 1) * R],
            lambda k: wtiles[k // TK][:, (k % TK) * SL:(k % TK + 1) * SL],
            nk, btile, 0, SL,
        )
        act = lrelu_act(pt, SL)
        # transpose the [64, 512] slice to [128, 4*64] and AllGather it
        att = atp.tile([KC, 4 * R], bf16, tag="atT")
        for j in range(4):
            tp = pst.tile([KC, R], bf16, tag="pst")
            nc.tensor.transpose(tp[:], act[:, j * KC:(j + 1) * KC], id64[:])
            dst = att[:, j * R:(j + 1) * R]
            if j % 2 == 0:
                nc.vector.tensor_copy(dst, tp[:])
            else:
                nc.scalar.activation(dst, tp[:], AF.Copy)
        ag_in = dram.tile([KC, 4 * R], bf16, tag="agin")
        nc.scalar.dma_start(ag_in[:], att[:])
        ag_out = dram.tile([NC * KC, 4 * R], bf16, tag="agout",
                           addr_space="Shared")
        nc.gpsimd.collective_compute(
            "AllGather", ALU.bypass, replica_groups=rg,
            ins=[ag_in[:].opt()], outs=[ag_out[:].opt()],
        )
        # per-rank contiguous loads back to SBUF, spread across queues
        lt = lpool.tile([KC, NC * 4 * R], bf16, tag="lhs")
        for r in range(NC):
            src = ag_out[r * KC:(r + 1) * KC, :]
            dst = lt[:, r * 4 * R:(r + 1) * 4 * R]
            eng = (nc.scalar, nc.sync, nc.gpsimd)[r % 3]
            eng.dma_start(dst, src)

    # ---- layer 6: full W6 on every core (no collective), K=4096, N=1024 ----
    nk6 = H // KC
    b6t = bpool.tile([1, OF], bf16, tag="b6")
    nc.sync.dma_start(b6t[0:1, :], aps["b6"].unsqueeze(0))
    M = pipool.tile([R, OF], f32, tag="M")
    for nb in range(2):
        w6tiles = load_w(aps[f"W6{'ab'[nb]}"], nk6, SL)
        pt = ps.tile([2 * R, SL], f32, tag="ps")
        mm_layer(
            pt,
            lambda k: lt[:, k * R:(k + 1) * R],
            lambda k: w6tiles[k // TK][:, (k % TK) * SL:(k % TK + 1) * SL],
            nk6, b6t, nb * SL, SL,
        )
        z = sum_halves(pt, SL)
        lr = apool.tile([R, SL], f32, tag="lrelu_out")
        nc.scalar.activation(lr[:], z[:], AF.Lrelu, alpha=SLOPE)
        sg = apool.tile([R, SL], f32, tag="sig")
        nc.scalar.activation(sg[:], lr[:], AF.Sigmoid)
        nc.vector.tensor_scalar(
            M[:, nb * SL:(nb + 1) * SL], sg[:], mac[:, 0:1], mac[:, 1:2],
            op0=ALU.mult, op1=ALU.add,
        )

    # ---- power iteration: b <- M b, unnormalized ----
    # each mult+segmented-reduce step is split between the Vector and GpSimd
    # engines (disjoint r-blocks) to halve the serial latency
    RS = 20  # rows 0:RS on vector, RS:32 on gpsimd (gpsimd is slower)
    M3 = M[:].rearrange("p (r q) -> p r q", q=N)
    Mb = pipool.tile([R, OF], bf16, tag="Mb")
    nc.scalar.activation(Mb[:], M[:], AF.Copy)
    Mb3 = Mb[:].rearrange("p (r q) -> p r q", q=N)
    bv = pipool.tile([R, N], f32, tag="bv")
    nc.vector.reduce_sum(bv[:], M3, axis=AX.X)  # first step from b0 = ones
    for it in range(PI_ITERS):
        last = it == PI_ITERS - 1
        bb = bv[:].unsqueeze(1).broadcast_to((R, N, N))
        if last:
            tmp = pipool.tile([R, OF], f32, tag="pit")
            t3 = tmp[:].rearrange("p (r q) -> p r q", q=N)
            src3 = M3
        else:
            tmp = pipool.tile([R, OF], bf16, tag="pitb")
            t3 = tmp[:].rearrange("p (r q) -> p r q", q=N)
            src3 = Mb3
        nc.vector.tensor_tensor(t3[:, 0:RS], src3[:, 0:RS], bb[:, 0:RS],
                                op=ALU.mult)
        nc.gpsimd.tensor_tensor(t3[:, RS:N], src3[:, RS:N], bb[:, RS:N],
                                op=ALU.mult)
        bv = pipool.tile([R, N], f32, tag="bv")
        nc.vector.reduce_sum(bv[:, 0:RS], t3[:, 0:RS], axis=AX.X)
        nc.vector.reduce_sum(bv[:, RS:N], t3[:, RS:N], axis=AX.X)

    # ---- deltas tail ----
    scr = tailp.tile([R, N], f32, tag="scr")
    d = tailp.tile([R, 1], f32, tag="d")
    nc.vector.tensor_tensor(scr[:], bv[:], dmask[:], op=ALU.mult)
    nc.vector.reduce_sum(d[:], scr[:], axis=AX.X)
    recipd = tailp.tile([R, 1], f32, tag="rd")
    nc.vector.reciprocal(recipd[:], d[:])
    recipE = tailp.tile([R, N], f32, tag="rE")
    nc.vector.reciprocal(recipE[:], bv[:])
    coef_s = tailp.tile([R, 1], f32, tag="cs")
    nc.vector.tensor_tensor(coef_s[:], t01[:, 0:1], recipd[:], op=ALU.mult)
    scr2 = tailp.tile([R, N], f32, tag="scr2")
    c23 = tailp.tile([R, 1], f32, tag="c23")
    nc.vector.tensor_tensor(scr2[:], tt23[:], recipE[:], op=ALU.mult)
    nc.vector.reduce_sum(c23[:], scr2[:], axis=AX.X)
    coef = tailp.tile([R, B], f32, tag="coef")
    nc.vector.memset(coef[:], 0.0)
    nc.vector.tensor_copy(coef[0:32, 0:1], coef_s[0:32, :])
    nc.vector.tensor_copy(coef[32:64, 1:2], coef_s[32:64, :])
    nc.vector.tensor_copy(coef[0:32, 2:3], c23[0:32, :])
    nc.vector.tensor_copy(coef[32:64, 3:4], c23[32:64, :])
    pd = pst.tile([B, N], f32, tag="pd", bufs=1)
    nc.tensor.matmul(pd[:], coef[:], bv[:], start=True, stop=True)
    osb = tailp.tile([B, N], f32, tag="osb")
    nc.vector.tensor_copy(osb[:], pd[:])
    nc.sync.dma_start(aps["out"][:], osb[:])
    es.close()


def build():
    import concourse.bacc as bacc
    import concourse.mybir as mybir
    import concourse.tile as tile

    f32 = mybir.dt.float32
    bf16 = mybir.dt.bfloat16
    nc = bacc.Bacc("TRN2", target_bir_lowering=False, debug=False, num_devices=NC)
    shapes = {
        "XT": ([KC, 2 * R], bf16),
        "W1": ([KC, 2 * H], bf16), "b1": ([H], bf16),
        "W2": ([KC, H // KC * SL], bf16), "b2": ([SL], bf16),
        "W3": ([KC, H // KC * SL], bf16), "b3": ([SL], bf16),
        "W4": ([KC, H // KC * SL], bf16), "b4": ([SL], bf16),
        "W5": ([KC, H // KC * SL], bf16), "b5": ([SL], bf16),
        "W6a": ([KC, H // KC * SL], bf16), "W6b": ([KC, H // KC * SL], bf16),
        "b6": ([OF], bf16),
        "T01": ([R, 1], f32), "TT23": ([R, N], f32),
        "DMASK": ([R, N], f32), "MAC": ([R, 2], f32), "ID64": ([64, 64], bf16),
        "WARMIN": ([KC, 8], bf16),
    }
    aps = {
        k: nc.dram_tensor(k, v[0], v[1], kind="ExternalInput").ap()
        for k, v in shapes.items()
    }
    aps["out"] = nc.dram_tensor("out", [B, N], f32, kind="ExternalOutput").ap()
    aps["warm"] = nc.dram_tensor("warm", [1, 8], bf16, kind="ExternalOutput").ap()
    with tile.TileContext(nc) as tc:
        _build_body(nc, tc, tile, mybir, aps)
    nc.compile()
    return nc


def _chunk_major(W):
    """[K, width] -> [128, (K//128)*width]; chunk k's rows land contiguous."""
    K, width = W.shape
    nk = K // KC
    return np.ascontiguousarray(
        W.reshape(nk, KC, width).transpose(1, 0, 2).reshape(KC, nk * width)
    )


def prep_in_maps(inputs):
    import ml_dtypes
    f = np.float32
    bf = ml_dtypes.bfloat16
    E = np.asarray(inputs["batch_node_embeddings"], f)   # (B,N,D)
    T = np.asarray(inputs["batch_Ts"], f)                # (B,N,N)
    mult = np.asarray(inputs["mult_const_batch"], f).reshape(-1)[0]
    add = np.asarray(inputs["add_const_batch"], f).reshape(-1)[0]
    S = np.transpose(E, (1, 0, 2))                       # (N,B,D)
    G0 = np.concatenate([S[:, 0], S[:, 1]], axis=-1)     # (N, 2D)
    G1 = np.concatenate([S[:, 2], S[:, 3]], axis=-1)
    rows = np.concatenate([G0, G1], axis=0)              # (64, 256)
    XT = np.ascontiguousarray(rows.T)                    # (256, 64)
    W6 = np.asarray(inputs["W6"], f)
    common = {
        "XT": _chunk_major(XT).astype(bf),
        "W1": _chunk_major(np.asarray(inputs["W1"], f)).astype(bf),
        "b1": np.asarray(inputs["b1"], f).astype(bf),
        "W6a": _chunk_major(W6[:, 0:512]).astype(bf),
        "W6b": _chunk_major(W6[:, 512:1024]).astype(bf),
        "b6": np.asarray(inputs["b6"], f).astype(bf),
        "T01": np.ascontiguousarray(
            np.concatenate([T[0], T[1]], axis=0).sum(axis=1, keepdims=True)
        ),
        "TT23": np.ascontiguousarray(np.concatenate([T[2].T, T[3].T], axis=0)),
        "DMASK": np.ascontiguousarray(np.tile(np.eye(N, dtype=f), (2, 1))),
        "MAC": np.ascontiguousarray(
            np.stack([np.full(R, mult, f), np.full(R, add, f)], axis=1)
        ),
        "ID64": np.eye(64, dtype=bf),
        "WARMIN": np.zeros((KC, 8), bf),
    }
    in_maps = []
    for c in range(NC):
        m = dict(common)
        for li in range(2, 6):
            W = np.asarray(inputs[f"W{li}"], f)
            b = np.asarray(inputs[f"b{li}"], f)
            m[f"W{li}"] = _chunk_major(W[:, c * SL:(c + 1) * SL]).astype(bf)
            m[f"b{li}"] = np.ascontiguousarray(b[c * SL:(c + 1) * SL]).astype(bf)
        in_maps.append(m)
    return in_maps


def kernel(**inputs):
    global _COMPILED, LAST_RESULTS
    from concourse import bass_utils

    if _COMPILED is None:
        _COMPILED = build()
    in_maps = prep_in_maps(inputs)
    res = bass_utils.run_bass_kernel_spmd(
        _COMPILED, in_maps, core_ids=list(range(NC))
    )
    LAST_RESULTS = res
    return np.asarray(res.results[0]["out"], np.float32)
